# revision 20
# baseline (speedup 1.0000x reference)
"""Trainium2 Bass kernel for nn_AudioDeviceModel (dense_cnn, memory-bound).

The reference model applies a chain of dilated kernel-size-2 convs to a
length-1 sequence with SAME padding.  For dilation d the two taps land at
padded positions 0 and d while the real sample sits at position d//2, so
every conv after the first reduces to its bias; the first conv (dilation 1,
pad_low=0) reduces to tap 0: a dot product of x[b, :] with w1[0, :, 0].
The whole model is therefore

    out[b, j] = (x[b, :] . w1[0, :, 0]) * wd[0, j] + bd_eff[j]
    bd_eff[j] = (b1 + b2 + b3 + b4 + b5) * wd[0, j] + bd[j]

(verified numerically against the jax reference).  A pure memory-bound
row-wise dot product over a 512 MiB matrix, data-parallel over 8 cores.

x and v are staged to HBM in bf16 (fp32 accumulation on device; max rel
err ~2e-3 vs the 2e-2 gate), so HBM traffic is ~34 MB/core: a ~95 us DMA
floor at the measured 356-384 GB/s.  Engine facts measured on HW here:
scalar_tensor_tensor and ACT activation run at 1x (no bf16 speedup);
tensor_tensor runs at 2x; PE rank-1 broadcast matmuls cost ~0.77 us per
512 cols.  No single engine can keep up with the stream, so the
multiply+reduce is spread:

- 'stt' chunks: DVE scalar_tensor_tensor (1x, single hop).
- 'tt' chunks: DVE tensor_tensor multiply (2x) + ACT activation(Copy,
  accum_out) row-sum (1x).
(GpSimd cannot help: the v3 ISA restricts Pool tensor_reduce and
TensorScalarPtr to int ops, so there is no third float reduce engine.)

The last three row-blocks alternate stt/tt at 4096 granularity so DVE
and ACT drain the stream tail in parallel.  Block 0 starts with 2048/
4096 chunks fed by a stride-0 DMA broadcast of v[0:8192] (compute
starts ~8 us); v[8192:] is broadcast on-chip (PE ones.T @ v_slice ->
PSUM -> ACT copy) to keep DMA bytes minimal.

DMA topology: x tiles go 2:1 to the sync and scalar HWDGE rings (mixing
in the gpsimd SWDGE ring for x measured 276 GB/s vs 356+ for pure
HWDGE).  Every chunk's DMA is issued PREFETCH chunks ahead of its
compute in program order, so ring doorbells are never queued behind
multi-us ACT/DVE ops (the v5 convoy cost ~25 us).  Out-DMAs are emitted
on the sync ring OUT_DELAY blocks late for the same reason.

This container's walrus build only accepts ONE on_wait and ONE on_update
per instruction, while Tile emits multi-wait instructions (kernel-tail
drain, multi-dependency compute ops).  legalize_bir_sync() splits the
extras into standalone EventSemaphore/NoOp instructions on the same engine
(sequencers are in-order, so a wait immediately before an instruction is
equivalent; trailing updates only on non-DMA instructions).
"""

import json

import ml_dtypes
import numpy as np

import concourse.bass as bass
import concourse.mybir as mybir
import concourse.tile as tile
from concourse.bass_utils import run_bass_kernel_spmd

FP32 = mybir.dt.float32
BF16 = mybir.dt.bfloat16

N_CORES = 8
B_FULL = 8192
L = 16384
J = 128
B_CORE = B_FULL // N_CORES  # 1024
P = 128                     # SBUF partitions
F = 8192                    # max L-chunk (free dim) per DMA tile
MM = 512                    # matmul moving free dim (PSUM bank)
V_DMA = 8192                # v[0:V_DMA] broadcast via DMA; rest via PE

PREFETCH = 4                # chunks of DMA lead over compute
OUT_DELAY = 3               # flush out-DMAs this many row-blocks late

# (offset, size, mode) per row-block; sizes sum to L.  Modes: 'stt' = DVE
# scalar_tensor_tensor (1x, single hop), 'tt' = DVE tensor_tensor (2x)
# + ACT activation accum (1x).  (GpSimd can't help: the v3 ISA has no
# float TensorScalarPtr or float tensor_reduce on Pool.)
CHUNK_TABLE = [
    [(0, 2048, 'stt'), (2048, 2048, 'stt'), (4096, 4096, 'stt'),
     (8192, 4096, 'tt'), (12288, 4096, 'tt')],                       # 0
    [(0, 8192, 'tt'), (8192, 8192, 'tt')],                           # 1
    [(0, 4096, 'stt'), (4096, 4096, 'tt'), (8192, 8192, 'tt')],      # 2
    [(0, 8192, 'tt'), (8192, 8192, 'tt')],                           # 3
    [(0, 8192, 'tt'), (8192, 8192, 'tt')],                           # 4
    [(0, 4096, 'stt'), (4096, 4096, 'tt'),
     (8192, 4096, 'stt'), (12288, 4096, 'tt')],                      # 5
    [(0, 4096, 'tt'), (4096, 4096, 'stt'),
     (8192, 4096, 'tt'), (12288, 4096, 'stt')],                      # 6
    [(0, 4096, 'stt'), (4096, 4096, 'tt'), (8192, 2048, 'stt'),
     (10240, 2048, 'tt'), (12288, 1024, 'stt'), (13312, 1024, 'tt'),
     (14336, 512, 'stt'), (14848, 1024, 'tt'), (15872, 512, 'stt')],  # 7
]
for _chunks in CHUNK_TABLE:
    assert sum(c[1] for c in _chunks) == L


def legalize_bir_sync(bir_bytes: bytes) -> bytes:
    """Split >1 on_wait / on_update per instruction for this walrus build."""
    mod = json.loads(bir_bytes)
    for fn in mod["functions"]:
        for bb in fn["blocks"]:
            out = []
            for ins in bb["instructions"]:
                si = ins.get("sync_info")
                waits = (si or {}).get("on_wait") or []
                ups = (si or {}).get("on_update") or []
                if len(waits) > 1:
                    for i, w in enumerate(waits[:-1]):
                        out.append({
                            "debug": ins.get("debug"),
                            "engine": ins["engine"],
                            "ins": [],
                            "outs": [],
                            "name": f"{ins['name']}_lw{i}",
                            "opcode": "EventSemaphore",
                            "sync_info": {"on_update": [], "on_wait": [w]},
                        })
                    si["on_wait"] = [waits[-1]]
                out.append(ins)
                if len(ups) > 1:
                    if ins.get("opcode") == "DMACopy":
                        raise RuntimeError(
                            f"multi-update on DMA {ins['name']} cannot be legalized"
                        )
                    for i, u in enumerate(ups[1:]):
                        out.append({
                            "debug": ins.get("debug"),
                            "engine": ins["engine"],
                            "ins": [],
                            "outs": [],
                            "name": f"{ins['name']}_lu{i}",
                            "opcode": "NoOp",
                            "sync_info": {"on_update": [u], "on_wait": []},
                        })
                    si["on_update"] = [ups[0]]
            bb["instructions"] = out
    return json.dumps(mod).encode()


def install_legalizer(nc):
    orig = nc.to_json_bytes

    def patched():
        return legalize_bir_sync(orig())

    nc.to_json_bytes = patched
    return nc


def build_module(b_core: int = B_CORE, l: int = L) -> bass.Bass:
    n_bb = b_core // P
    nc = bass.Bass()
    x_ds = [
        nc.dram_tensor(f"x{bb}", [P, l], BF16, kind="ExternalInput")
        for bb in range(n_bb)
    ]
    v_d = nc.dram_tensor("vb", [l], BF16, kind="ExternalInput")
    wd_d = nc.dram_tensor("wdrow", [J], FP32, kind="ExternalInput")
    bd_d = nc.dram_tensor("bdeff", [J], FP32, kind="ExternalInput")
    out_d = nc.dram_tensor("out", [b_core, J], FP32, kind="ExternalOutput")

    # flat chunk list: (bb, ci, off, f, mode, n_chunks_in_block)
    all_chunks = []
    for bb, chunks in enumerate(CHUNK_TABLE):
        for ci, (off, f, mode) in enumerate(chunks):
            all_chunks.append((bb, ci, off, f, mode, len(chunks)))
    n_idx = len(all_chunks)

    with tile.TileContext(nc) as tc:
        with (
            tc.tile_pool(name="consts", bufs=1) as consts,
            tc.tile_pool(name="xp", bufs=7) as xp,
            tc.tile_pool(name="accp", bufs=20) as accp,
            tc.tile_pool(name="outp", bufs=5) as outp,
        ):
            # v row for the PE broadcast: on gpsimd so its completion
            # semaphore is independent of the x-tile HWDGE lanes.
            vrow = consts.tile([1, l], BF16, name="vrow", tag="vrow")
            nc.gpsimd.dma_start(out=vrow, in_=v_d[:].unsqueeze(0))

            ones = consts.tile([1, P], BF16, name="ones", tag="ones")
            nc.vector.memset(ones, 1.0)

            wd_b = consts.tile([P, J], FP32, name="wd_b", tag="wd_b")
            nc.gpsimd.dma_start(
                out=wd_b, in_=wd_d[:].unsqueeze(0).partition_broadcast(P)
            )
            bd_b = consts.tile([P, J], FP32, name="bd_b", tag="bd_b")
            nc.gpsimd.dma_start(
                out=bd_b, in_=bd_d[:].unsqueeze(0).partition_broadcast(P)
            )

            v_bs = [
                consts.tile([P, F], BF16, name=f"vb{c}", tag=f"vb{c}")
                for c in range(l // F)
            ]
            # v[0:V_DMA]: stride-0 DMA broadcast, first on the scalar ring,
            # in 2048-col pieces so block 0's early chunks unblock ASAP.
            for r in range(V_DMA // 2048):
                nc.scalar.dma_start(
                    out=v_bs[0][:, r * 2048:(r + 1) * 2048],
                    in_=v_d[r * 2048:(r + 1) * 2048]
                    .unsqueeze(0).partition_broadcast(P),
                )

            def v_slice(off, f):
                c0, k0 = divmod(off, F)
                return v_bs[c0][:, k0:k0 + f]

            with tc.tile_pool(name="psum", bufs=4, space="PSUM") as psum:
                # v[V_DMA:]: PE rank-1 broadcast + ACT PSUM->SBUF copies.
                for g in range((l - V_DMA) // (2 * MM)):
                    pt = psum.tile([P, 2 * MM], FP32)
                    for h in range(2):
                        k = V_DMA + (2 * g + h) * MM
                        nc.tensor.matmul(
                            pt[:, h * MM:(h + 1) * MM],
                            ones,
                            vrow[:, k:k + MM],
                            start=True, stop=True,
                        )
                    c, kk = divmod(V_DMA + 2 * g * MM, F)
                    nc.scalar.activation(
                        out=v_bs[c][:, kk:kk + 2 * MM], in_=pt,
                        func=mybir.ActivationFunctionType.Copy,
                    )

                rings = (nc.sync, nc.sync, nc.scalar)  # 2:1 split
                x_tiles = {}
                accs = {}
                pending_outs = []   # (bb, o_t)

                def issue_dma(idx):
                    bb, ci, off, f, mode, _ = all_chunks[idx]
                    x_t = xp.tile([P, F], BF16, name="x_t", tag="x_t")
                    x_tiles[idx] = x_t
                    rings[idx % 3].dma_start(
                        out=x_t[:, :f], in_=x_ds[bb][:, off:off + f]
                    )

                def flush_outs(upto_bb):
                    while pending_outs and pending_outs[0][0] <= upto_bb:
                        obb, o_t = pending_outs.pop(0)
                        nc.sync.dma_start(
                            out=out_d[obb * P:(obb + 1) * P, :], in_=o_t
                        )

                def emit_epilogue(bb):
                    nch = len(CHUNK_TABLE[bb])
                    t = accp.tile([P, 1], FP32, name=f"t{bb}", tag=f"t{bb}")
                    nc.vector.tensor_reduce(
                        out=t, in_=accs[bb], axis=mybir.AxisListType.X,
                        op=mybir.AluOpType.add,
                    )
                    o_t = outp.tile([P, J], FP32)
                    nc.vector.scalar_tensor_tensor(
                        out=o_t, in0=wd_b, scalar=t, in1=bd_b,
                        op0=mybir.AluOpType.mult, op1=mybir.AluOpType.add,
                    )
                    pending_outs.append((bb, o_t))

                for idx in range(min(PREFETCH, n_idx)):
                    issue_dma(idx)

                for idx, (bb, ci, off, f, mode, nch) in enumerate(all_chunks):
                    if idx + PREFETCH < n_idx:
                        nbb = all_chunks[idx + PREFETCH][0]
                        flush_outs(nbb - OUT_DELAY)
                        issue_dma(idx + PREFETCH)
                    if bb not in accs:
                        accs[bb] = accp.tile(
                            [P, nch], FP32, name=f"acc{bb}", tag=f"acc{bb}"
                        )
                    x_t = x_tiles.pop(idx)
                    acc_col = accs[bb][:, ci:ci + 1]
                    if mode == 'stt':
                        nc.vector.scalar_tensor_tensor(
                            out=x_t[:, :f], in0=x_t[:, :f], scalar=1.0,
                            in1=v_slice(off, f),
                            op0=mybir.AluOpType.mult,
                            op1=mybir.AluOpType.mult,
                            accum_out=acc_col,
                        )
                    else:
                        nc.vector.tensor_tensor(
                            out=x_t[:, :f], in0=x_t[:, :f],
                            in1=v_slice(off, f), op=mybir.AluOpType.mult,
                        )
                        nc.scalar.activation(
                            out=x_t[:, :f], in_=x_t[:, :f],
                            func=mybir.ActivationFunctionType.Copy,
                            accum_out=acc_col,
                        )
                    if ci == nch - 1:
                        emit_epilogue(bb)
                flush_outs(n_bb)
    install_legalizer(nc)
    return nc


_module_cache: dict = {}


def get_module() -> bass.Bass:
    if "nc" not in _module_cache:
        _module_cache["nc"] = build_module()
    return _module_cache["nc"]


def make_in_maps(inputs: dict) -> list[dict]:
    """Shard the full inputs into one input map per core (pure data parallel
    on the batch dim; tiny weights replicated).  x and v are staged in bf16
    (fp32 accumulation on device keeps the result inside the error gate)."""
    x = np.asarray(inputs["x"], dtype=np.float32)
    xb = np.ascontiguousarray(x).astype(ml_dtypes.bfloat16)
    w1 = np.asarray(inputs["w1"], dtype=np.float32)
    vb = np.ascontiguousarray(w1[0, :, 0]).astype(ml_dtypes.bfloat16)
    s0 = float(sum(
        np.asarray(inputs[k], np.float32).reshape(-1)[0]
        for k in ("b1", "b2", "b3", "b4", "b5")
    ))
    wd_row = np.ascontiguousarray(np.asarray(inputs["wd"], np.float32)[0, :])
    bd = np.asarray(inputs["bd"], np.float32).reshape(-1)
    bd_eff = np.ascontiguousarray((s0 * wd_row + bd).astype(np.float32))

    maps = []
    for c in range(N_CORES):
        m = {"vb": vb, "wdrow": wd_row, "bdeff": bd_eff}
        base = c * B_CORE
        for bb in range(B_CORE // P):
            m[f"x{bb}"] = np.ascontiguousarray(
                xb[base + bb * P:base + (bb + 1) * P]
            )
        maps.append(m)
    return maps


def kernel(**inputs) -> np.ndarray:
    nc = get_module()
    in_maps = make_in_maps(inputs)
    res = run_bass_kernel_spmd(nc, in_maps, core_ids=list(range(N_CORES)))
    return np.concatenate([r["out"] for r in res.results], axis=0)


# revision 24
# speedup vs baseline: 1.0294x; 1.0294x over previous
"""Trainium2 Bass kernel for nn_AudioDeviceModel (dense_cnn, memory-bound).

The reference model applies a chain of dilated kernel-size-2 convs to a
length-1 sequence with SAME padding.  For dilation d the two taps land at
padded positions 0 and d while the real sample sits at position d//2, so
every conv after the first reduces to its bias; the first conv (dilation 1,
pad_low=0) reduces to tap 0: a dot product of x[b, :] with w1[0, :, 0].
The whole model is therefore

    out[b, j] = (x[b, :] . w1[0, :, 0]) * wd[0, j] + bd_eff[j]
    bd_eff[j] = (b1 + b2 + b3 + b4 + b5) * wd[0, j] + bd[j]

(verified numerically against the jax reference).  A pure memory-bound
row-wise dot product over a 512 MiB matrix, data-parallel over 8 cores.

x and v are staged to HBM in bf16 (fp32 accumulation on device; max rel
err ~2e-3 vs the 2e-2 gate), so per-core traffic is ~38 MB: 32 MiB of x,
a 4 MiB host-replicated v table ([128, L], a plain full-rate load -- the
stride-0 broadcast DMA measured ~170 GB/s and the PE rank-1 broadcast
~0.8 us per 512 cols, both worse), outputs and epilogue consts.

Engine facts measured on HW here: HWDGE rings burst 420+ GB/s when both
queues are fed; scalar_tensor_tensor and ACT activation run at 1x for
bf16; tensor_tensor runs at 2x.  No single engine keeps up with the
stream, so the multiply+reduce is split per chunk:

- 'stt' chunks: DVE scalar_tensor_tensor (1x, single hop).
- 'tt' chunks: DVE tensor_tensor multiply (2x) + ACT activation(Copy,
  accum_out) row-sum (1x).  DVE ~89 us, ACT ~85 us.
(GpSimd cannot help: the v3 ISA restricts Pool tensor_reduce and
TensorScalarPtr to int ops.)

Block 0 starts with 2048/4096 chunks so compute begins as soon as the
first v piece lands (~8 us); the last three row-blocks alternate stt/tt
at 4096 granularity so DVE and ACT drain the stream tail in parallel,
tapering to 512 so the post-last-byte tail is ~2 us.

DMA topology: x tiles alternate 1:1 between the sync and scalar HWDGE
rings (a 2:1 split starves one queue: a single queue sustains only
~280 GB/s; mixing the gpsimd SWDGE ring in measured 276 GB/s).  Every
chunk's DMA is issued PREFETCH chunks ahead of its compute in program
order so ring doorbells are never queued behind multi-us compute ops
(that convoy cost v5 ~25 us).  Out-DMAs are emitted on the sync ring
OUT_DELAY blocks late for the same reason.  The epilogue consts ride
the gpsimd SWDGE ring.

This container's walrus build only accepts ONE on_wait and ONE on_update
per instruction, while Tile emits multi-wait instructions (kernel-tail
drain, multi-dependency compute ops).  legalize_bir_sync() splits the
extras into standalone EventSemaphore/NoOp instructions on the same engine
(sequencers are in-order, so a wait immediately before an instruction is
equivalent; trailing updates only on non-DMA instructions).
"""

import json

import ml_dtypes
import numpy as np

import concourse.bass as bass
import concourse.mybir as mybir
import concourse.tile as tile
from concourse.bass_utils import run_bass_kernel_spmd

FP32 = mybir.dt.float32
BF16 = mybir.dt.bfloat16

N_CORES = 8
B_FULL = 8192
L = 16384
J = 128
B_CORE = B_FULL // N_CORES  # 1024
P = 128                     # SBUF partitions
F = 8192                    # max L-chunk (free dim) per DMA tile

PREFETCH = 4                # chunks of DMA lead over compute
OUT_DELAY = 3               # flush out-DMAs this many row-blocks late

# (offset, size, mode) per row-block; sizes sum to L.  'stt' = DVE
# scalar_tensor_tensor (1x); 'tt' = DVE tensor_tensor (2x) + ACT accum.
CHUNK_TABLE = [
    [(0, 2048, 'stt'), (2048, 2048, 'stt'), (4096, 4096, 'stt'),
     (8192, 4096, 'tt'), (12288, 4096, 'tt')],                       # 0
    [(0, 8192, 'tt'), (8192, 8192, 'tt')],                           # 1
    [(0, 4096, 'tt'), (4096, 4096, 'tt'), (8192, 8192, 'tt')],       # 2
    [(0, 8192, 'tt'), (8192, 8192, 'tt')],                           # 3
    [(0, 8192, 'tt'), (8192, 8192, 'tt')],                           # 4
    [(0, 4096, 'stt'), (4096, 4096, 'tt'),
     (8192, 4096, 'stt'), (12288, 4096, 'tt')],                      # 5
    [(0, 4096, 'tt'), (4096, 4096, 'stt'),
     (8192, 4096, 'tt'), (12288, 4096, 'stt')],                      # 6
    [(0, 4096, 'stt'), (4096, 4096, 'tt'), (8192, 2048, 'stt'),
     (10240, 2048, 'tt'), (12288, 1024, 'stt'), (13312, 1024, 'tt'),
     (14336, 512, 'stt'), (14848, 1024, 'tt'), (15872, 512, 'stt')],  # 7
]
for _chunks in CHUNK_TABLE:
    assert sum(c[1] for c in _chunks) == L


def legalize_bir_sync(bir_bytes: bytes) -> bytes:
    """Split >1 on_wait / on_update per instruction for this walrus build."""
    mod = json.loads(bir_bytes)
    for fn in mod["functions"]:
        for bb in fn["blocks"]:
            out = []
            for ins in bb["instructions"]:
                si = ins.get("sync_info")
                waits = (si or {}).get("on_wait") or []
                ups = (si or {}).get("on_update") or []
                if len(waits) > 1:
                    for i, w in enumerate(waits[:-1]):
                        out.append({
                            "debug": ins.get("debug"),
                            "engine": ins["engine"],
                            "ins": [],
                            "outs": [],
                            "name": f"{ins['name']}_lw{i}",
                            "opcode": "EventSemaphore",
                            "sync_info": {"on_update": [], "on_wait": [w]},
                        })
                    si["on_wait"] = [waits[-1]]
                out.append(ins)
                if len(ups) > 1:
                    if ins.get("opcode") == "DMACopy":
                        raise RuntimeError(
                            f"multi-update on DMA {ins['name']} cannot be legalized"
                        )
                    for i, u in enumerate(ups[1:]):
                        out.append({
                            "debug": ins.get("debug"),
                            "engine": ins["engine"],
                            "ins": [],
                            "outs": [],
                            "name": f"{ins['name']}_lu{i}",
                            "opcode": "NoOp",
                            "sync_info": {"on_update": [u], "on_wait": []},
                        })
                    si["on_update"] = [ups[0]]
            bb["instructions"] = out
    return json.dumps(mod).encode()


def install_legalizer(nc):
    orig = nc.to_json_bytes

    def patched():
        return legalize_bir_sync(orig())

    nc.to_json_bytes = patched
    return nc


def build_module(b_core: int = B_CORE, l: int = L) -> bass.Bass:
    n_bb = b_core // P
    nc = bass.Bass()
    x_ds = [
        nc.dram_tensor(f"x{bb}", [P, l], BF16, kind="ExternalInput")
        for bb in range(n_bb)
    ]
    v_d = nc.dram_tensor("vtab", [P, l], BF16, kind="ExternalInput")
    wd_d = nc.dram_tensor("wdrow", [J], FP32, kind="ExternalInput")
    bd_d = nc.dram_tensor("bdeff", [J], FP32, kind="ExternalInput")
    out_d = nc.dram_tensor("out", [b_core, J], FP32, kind="ExternalOutput")

    # flat chunk list: (bb, ci, off, f, mode, n_chunks_in_block)
    all_chunks = []
    for bb, chunks in enumerate(CHUNK_TABLE):
        for ci, (off, f, mode) in enumerate(chunks):
            all_chunks.append((bb, ci, off, f, mode, len(chunks)))
    n_idx = len(all_chunks)

    with tile.TileContext(nc) as tc:
        with (
            tc.tile_pool(name="consts", bufs=1) as consts,
            tc.tile_pool(name="xp", bufs=7) as xp,
            tc.tile_pool(name="accp", bufs=20) as accp,
            tc.tile_pool(name="outp", bufs=5) as outp,
        ):
            rings = (nc.sync, nc.scalar)

            # v table (host-replicated to all 128 partitions): four 1 MiB
            # loads, two per ring, ahead of the x tiles.  v[0:4096] lands
            # first so block 0's early chunks unblock ~8 us in.
            v_full = consts.tile([P, l], BF16, name="v_full", tag="v_full")
            for r in range(4):
                rings[r % 2].dma_start(
                    out=v_full[:, r * 4096:(r + 1) * 4096],
                    in_=v_d[:, r * 4096:(r + 1) * 4096],
                )

            # tiny epilogue consts on the gpsimd (SWDGE) ring.
            wd_b = consts.tile([P, J], FP32, name="wd_b", tag="wd_b")
            nc.gpsimd.dma_start(
                out=wd_b, in_=wd_d[:].unsqueeze(0).partition_broadcast(P)
            )
            bd_b = consts.tile([P, J], FP32, name="bd_b", tag="bd_b")
            nc.gpsimd.dma_start(
                out=bd_b, in_=bd_d[:].unsqueeze(0).partition_broadcast(P)
            )

            x_tiles = {}
            accs = {}
            pending_outs = []   # (bb, o_t)

            def issue_dma(idx):
                bb, ci, off, f, mode, _ = all_chunks[idx]
                x_t = xp.tile([P, F], BF16, name="x_t", tag="x_t")
                x_tiles[idx] = x_t
                rings[idx % 2].dma_start(
                    out=x_t[:, :f], in_=x_ds[bb][:, off:off + f]
                )

            def flush_outs(upto_bb):
                while pending_outs and pending_outs[0][0] <= upto_bb:
                    obb, o_t = pending_outs.pop(0)
                    nc.sync.dma_start(
                        out=out_d[obb * P:(obb + 1) * P, :], in_=o_t
                    )

            def emit_epilogue(bb):
                t = accp.tile([P, 1], FP32, name=f"t{bb}", tag=f"t{bb}")
                nc.vector.tensor_reduce(
                    out=t, in_=accs[bb], axis=mybir.AxisListType.X,
                    op=mybir.AluOpType.add,
                )
                o_t = outp.tile([P, J], FP32)
                nc.vector.scalar_tensor_tensor(
                    out=o_t, in0=wd_b, scalar=t, in1=bd_b,
                    op0=mybir.AluOpType.mult, op1=mybir.AluOpType.add,
                )
                pending_outs.append((bb, o_t))

            for idx in range(min(PREFETCH, n_idx)):
                issue_dma(idx)

            for idx, (bb, ci, off, f, mode, nch) in enumerate(all_chunks):
                if idx + PREFETCH < n_idx:
                    nbb = all_chunks[idx + PREFETCH][0]
                    flush_outs(nbb - OUT_DELAY)
                    issue_dma(idx + PREFETCH)
                if bb not in accs:
                    accs[bb] = accp.tile(
                        [P, nch], FP32, name=f"acc{bb}", tag=f"acc{bb}"
                    )
                x_t = x_tiles.pop(idx)
                acc_col = accs[bb][:, ci:ci + 1]
                v_sl = v_full[:, off:off + f]
                if mode == 'stt':
                    nc.vector.scalar_tensor_tensor(
                        out=x_t[:, :f], in0=x_t[:, :f], scalar=1.0,
                        in1=v_sl,
                        op0=mybir.AluOpType.mult,
                        op1=mybir.AluOpType.mult,
                        accum_out=acc_col,
                    )
                else:
                    nc.vector.tensor_tensor(
                        out=x_t[:, :f], in0=x_t[:, :f],
                        in1=v_sl, op=mybir.AluOpType.mult,
                    )
                    nc.scalar.activation(
                        out=x_t[:, :f], in_=x_t[:, :f],
                        func=mybir.ActivationFunctionType.Copy,
                        accum_out=acc_col,
                    )
                if ci == nch - 1:
                    emit_epilogue(bb)
            flush_outs(n_bb)
    install_legalizer(nc)
    return nc


_module_cache: dict = {}


def get_module() -> bass.Bass:
    if "nc" not in _module_cache:
        _module_cache["nc"] = build_module()
    return _module_cache["nc"]


def make_in_maps(inputs: dict) -> list[dict]:
    """Shard the full inputs into one input map per core (pure data parallel
    on the batch dim; tiny weights replicated).  x and v are staged in bf16
    (fp32 accumulation on device keeps the result inside the error gate);
    v is host-replicated to [128, L] so the on-device load is a plain
    full-rate DMA."""
    x = np.asarray(inputs["x"], dtype=np.float32)
    xb = np.ascontiguousarray(x).astype(ml_dtypes.bfloat16)
    w1 = np.asarray(inputs["w1"], dtype=np.float32)
    vb = np.ascontiguousarray(w1[0, :, 0]).astype(ml_dtypes.bfloat16)
    vtab = np.ascontiguousarray(np.broadcast_to(vb[None, :], (P, L)))
    s0 = float(sum(
        np.asarray(inputs[k], np.float32).reshape(-1)[0]
        for k in ("b1", "b2", "b3", "b4", "b5")
    ))
    wd_row = np.ascontiguousarray(np.asarray(inputs["wd"], np.float32)[0, :])
    bd = np.asarray(inputs["bd"], np.float32).reshape(-1)
    bd_eff = np.ascontiguousarray((s0 * wd_row + bd).astype(np.float32))

    maps = []
    for c in range(N_CORES):
        m = {"vtab": vtab, "wdrow": wd_row, "bdeff": bd_eff}
        base = c * B_CORE
        for bb in range(B_CORE // P):
            m[f"x{bb}"] = np.ascontiguousarray(
                xb[base + bb * P:base + (bb + 1) * P]
            )
        maps.append(m)
    return maps


def kernel(**inputs) -> np.ndarray:
    nc = get_module()
    in_maps = make_in_maps(inputs)
    res = run_bass_kernel_spmd(nc, in_maps, core_ids=list(range(N_CORES)))
    return np.concatenate([r["out"] for r in res.results], axis=0)


# revision 29
# speedup vs baseline: 1.0354x; 1.0058x over previous
"""Trainium2 Bass kernel for nn_AudioDeviceModel (dense_cnn, memory-bound).

The reference model applies a chain of dilated kernel-size-2 convs to a
length-1 sequence with SAME padding.  For dilation d the two taps land at
padded positions 0 and d while the real sample sits at position d//2, so
every conv after the first reduces to its bias; the first conv (dilation 1,
pad_low=0) reduces to tap 0: a dot product of x[b, :] with w1[0, :, 0].
The whole model is therefore

    out[b, j] = (x[b, :] . w1[0, :, 0]) * wd[0, j] + bd_eff[j]
    bd_eff[j] = (b1 + b2 + b3 + b4 + b5) * wd[0, j] + bd[j]

(verified numerically against the jax reference).  A pure memory-bound
row-wise dot product over a 512 MiB matrix, data-parallel over 8 cores.

x and v are staged to HBM in bf16 (fp32 accumulation on device; max rel
err ~2e-3 vs the 2e-2 gate), so per-core traffic is ~38 MB: 32 MiB of x,
a 4 MiB host-replicated v table ([128, L], a plain full-rate load -- the
stride-0 broadcast DMA measured ~170 GB/s and the PE rank-1 broadcast
~0.8 us per 512 cols, both worse), outputs and epilogue consts.

Engine facts measured on HW here: HWDGE rings burst 420+ GB/s when both
queues are fed; scalar_tensor_tensor and ACT activation run at 1x for
bf16; tensor_tensor runs at 2x.  No single engine keeps up with the
stream, so the multiply+reduce is split per chunk:

- 'stt' chunks: DVE scalar_tensor_tensor (1x, single hop).
- 'tt' chunks: DVE tensor_tensor multiply (2x) + ACT activation(Copy,
  accum_out) row-sum (1x).  DVE ~89 us, ACT ~85 us.
(GpSimd cannot help: the v3 ISA restricts Pool tensor_reduce and
TensorScalarPtr to int ops.)

Block 0 starts with 2048/4096 chunks so compute begins as soon as the
first v piece lands (~8 us); the last three row-blocks alternate stt/tt
at 4096 granularity so DVE and ACT drain the stream tail in parallel,
tapering to 512 so the post-last-byte tail is ~2 us.

DMA topology: x tiles alternate 1:1 between the sync and scalar HWDGE
rings (a 2:1 split starves one queue: a single queue sustains only
~280 GB/s; mixing the gpsimd SWDGE ring in measured 276 GB/s).  Every
chunk's DMA is issued PREFETCH chunks ahead of its compute in program
order so ring doorbells are never queued behind multi-us compute ops
(that convoy cost v5 ~25 us).  Out-DMAs are emitted on the sync ring
OUT_DELAY blocks late for the same reason.  The epilogue consts ride
the gpsimd SWDGE ring.

This container's walrus build only accepts ONE on_wait and ONE on_update
per instruction, while Tile emits multi-wait instructions (kernel-tail
drain, multi-dependency compute ops).  legalize_bir_sync() splits the
extras into standalone EventSemaphore/NoOp instructions on the same engine
(sequencers are in-order, so a wait immediately before an instruction is
equivalent; trailing updates only on non-DMA instructions).
"""

import json

import ml_dtypes
import numpy as np

import concourse.bass as bass
import concourse.mybir as mybir
import concourse.tile as tile
from concourse.bass_utils import run_bass_kernel_spmd

FP32 = mybir.dt.float32
BF16 = mybir.dt.bfloat16

N_CORES = 8
B_FULL = 8192
L = 16384
J = 128
B_CORE = B_FULL // N_CORES  # 1024
P = 128                     # SBUF partitions
F = 8192                    # max L-chunk (free dim) per DMA tile

PREFETCH = 7                # chunks of DMA lead over compute
OUT_DELAY = 3               # flush out-DMAs this many row-blocks late

# (offset, size, mode) per row-block; sizes sum to L.  'stt' = DVE
# scalar_tensor_tensor (1x); 'tt' = DVE tensor_tensor (2x) + ACT accum.
CHUNK_TABLE = [
    [(0, 2048, 'stt'), (2048, 2048, 'stt'), (4096, 4096, 'stt'),
     (8192, 4096, 'tt'), (12288, 4096, 'tt')],                       # 0
    [(0, 8192, 'tt'), (8192, 8192, 'tt')],                           # 1
    [(0, 4096, 'stt'), (4096, 4096, 'tt'), (8192, 8192, 'tt')],      # 2
    [(0, 8192, 'tt'), (8192, 8192, 'tt')],                           # 3
    [(0, 8192, 'tt'), (8192, 8192, 'tt')],                           # 4
    [(0, 4096, 'stt'), (4096, 4096, 'tt'),
     (8192, 4096, 'stt'), (12288, 4096, 'tt')],                      # 5
    [(0, 4096, 'tt'), (4096, 4096, 'stt'),
     (8192, 4096, 'tt'), (12288, 4096, 'stt')],                      # 6
    [(0, 4096, 'stt'), (4096, 4096, 'tt'), (8192, 2048, 'stt'),
     (10240, 2048, 'tt'), (12288, 1024, 'stt'), (13312, 1024, 'tt'),
     (14336, 512, 'stt'), (14848, 1024, 'tt'), (15872, 512, 'stt')],  # 7
]
for _chunks in CHUNK_TABLE:
    assert sum(c[1] for c in _chunks) == L


def legalize_bir_sync(bir_bytes: bytes) -> bytes:
    """Split >1 on_wait / on_update per instruction for this walrus build."""
    mod = json.loads(bir_bytes)
    for fn in mod["functions"]:
        for bb in fn["blocks"]:
            out = []
            for ins in bb["instructions"]:
                si = ins.get("sync_info")
                waits = (si or {}).get("on_wait") or []
                ups = (si or {}).get("on_update") or []
                if len(waits) > 1:
                    for i, w in enumerate(waits[:-1]):
                        out.append({
                            "debug": ins.get("debug"),
                            "engine": ins["engine"],
                            "ins": [],
                            "outs": [],
                            "name": f"{ins['name']}_lw{i}",
                            "opcode": "EventSemaphore",
                            "sync_info": {"on_update": [], "on_wait": [w]},
                        })
                    si["on_wait"] = [waits[-1]]
                out.append(ins)
                if len(ups) > 1:
                    if ins.get("opcode") == "DMACopy":
                        raise RuntimeError(
                            f"multi-update on DMA {ins['name']} cannot be legalized"
                        )
                    for i, u in enumerate(ups[1:]):
                        out.append({
                            "debug": ins.get("debug"),
                            "engine": ins["engine"],
                            "ins": [],
                            "outs": [],
                            "name": f"{ins['name']}_lu{i}",
                            "opcode": "NoOp",
                            "sync_info": {"on_update": [u], "on_wait": []},
                        })
                    si["on_update"] = [ups[0]]
            bb["instructions"] = out
    return json.dumps(mod).encode()


def install_legalizer(nc):
    orig = nc.to_json_bytes

    def patched():
        return legalize_bir_sync(orig())

    nc.to_json_bytes = patched
    return nc


def build_module(b_core: int = B_CORE, l: int = L) -> bass.Bass:
    n_bb = b_core // P
    nc = bass.Bass()
    x_ds = [
        nc.dram_tensor(f"x{bb}", [P, l], BF16, kind="ExternalInput")
        for bb in range(n_bb)
    ]
    v_d = nc.dram_tensor("vtab", [P, l], BF16, kind="ExternalInput")
    wd_d = nc.dram_tensor("wdrow", [J], FP32, kind="ExternalInput")
    bd_d = nc.dram_tensor("bdeff", [J], FP32, kind="ExternalInput")
    out_d = nc.dram_tensor("out", [b_core, J], FP32, kind="ExternalOutput")

    # flat chunk list: (bb, ci, off, f, mode, n_chunks_in_block)
    all_chunks = []
    for bb, chunks in enumerate(CHUNK_TABLE):
        for ci, (off, f, mode) in enumerate(chunks):
            all_chunks.append((bb, ci, off, f, mode, len(chunks)))
    n_idx = len(all_chunks)

    with tile.TileContext(nc) as tc:
        with (
            tc.tile_pool(name="consts", bufs=1) as consts,
            tc.tile_pool(name="xp", bufs=10) as xp,
            tc.tile_pool(name="accp", bufs=20) as accp,
            tc.tile_pool(name="outp", bufs=5) as outp,
        ):
            # 2:1 sync-heavy split: the scalar sequencer also runs the ACT
            # accumulation, so it gets fewer (~12) DMA dispatches; deep
            # prefetch keeps its queue backlogged anyway.
            rings = (nc.sync, nc.sync, nc.scalar)

            # v table (host-replicated to all 128 partitions): v[0:8192]
            # as two 1 MiB loads up front (block 0's chunks unblock ~8 us
            # in); v[8192:] interleaved behind the first x tiles below so
            # it doesn't delay their completions.
            v_full = consts.tile([P, l], BF16, name="v_full", tag="v_full")
            pending_v = []
            for r in range(4):
                def load_v(r=r):
                    rings[r % 2].dma_start(
                        out=v_full[:, r * 4096:(r + 1) * 4096],
                        in_=v_d[:, r * 4096:(r + 1) * 4096],
                    )
                if r < 2:
                    load_v()
                else:
                    pending_v.append(load_v)

            # tiny epilogue consts on the gpsimd (SWDGE) ring.
            wd_b = consts.tile([P, J], FP32, name="wd_b", tag="wd_b")
            nc.gpsimd.dma_start(
                out=wd_b, in_=wd_d[:].unsqueeze(0).partition_broadcast(P)
            )
            bd_b = consts.tile([P, J], FP32, name="bd_b", tag="bd_b")
            nc.gpsimd.dma_start(
                out=bd_b, in_=bd_d[:].unsqueeze(0).partition_broadcast(P)
            )

            x_tiles = {}
            accs = {}
            pending_outs = []   # (bb, o_t)

            def issue_dma(idx):
                bb, ci, off, f, mode, _ = all_chunks[idx]
                x_t = xp.tile([P, F], BF16, name="x_t", tag="x_t")
                x_tiles[idx] = x_t
                rings[idx % 3].dma_start(
                    out=x_t[:, :f], in_=x_ds[bb][:, off:off + f]
                )
                if pending_v:
                    pending_v.pop(0)()

            def flush_outs(upto_bb):
                while pending_outs and pending_outs[0][0] <= upto_bb:
                    obb, o_t = pending_outs.pop(0)
                    nc.sync.dma_start(
                        out=out_d[obb * P:(obb + 1) * P, :], in_=o_t
                    )

            def emit_epilogue(bb):
                t = accp.tile([P, 1], FP32, name=f"t{bb}", tag=f"t{bb}")
                nc.vector.tensor_reduce(
                    out=t, in_=accs[bb], axis=mybir.AxisListType.X,
                    op=mybir.AluOpType.add,
                )
                o_t = outp.tile([P, J], FP32)
                nc.vector.scalar_tensor_tensor(
                    out=o_t, in0=wd_b, scalar=t, in1=bd_b,
                    op0=mybir.AluOpType.mult, op1=mybir.AluOpType.add,
                )
                pending_outs.append((bb, o_t))

            for idx in range(min(PREFETCH, n_idx)):
                issue_dma(idx)

            for idx, (bb, ci, off, f, mode, nch) in enumerate(all_chunks):
                if idx + PREFETCH < n_idx:
                    nbb = all_chunks[idx + PREFETCH][0]
                    flush_outs(nbb - OUT_DELAY)
                    issue_dma(idx + PREFETCH)
                if bb not in accs:
                    accs[bb] = accp.tile(
                        [P, nch], FP32, name=f"acc{bb}", tag=f"acc{bb}"
                    )
                x_t = x_tiles.pop(idx)
                acc_col = accs[bb][:, ci:ci + 1]
                v_sl = v_full[:, off:off + f]
                if mode == 'stt':
                    nc.vector.scalar_tensor_tensor(
                        out=x_t[:, :f], in0=x_t[:, :f], scalar=1.0,
                        in1=v_sl,
                        op0=mybir.AluOpType.mult,
                        op1=mybir.AluOpType.mult,
                        accum_out=acc_col,
                    )
                else:
                    nc.vector.tensor_tensor(
                        out=x_t[:, :f], in0=x_t[:, :f],
                        in1=v_sl, op=mybir.AluOpType.mult,
                    )
                    nc.scalar.activation(
                        out=x_t[:, :f], in_=x_t[:, :f],
                        func=mybir.ActivationFunctionType.Copy,
                        accum_out=acc_col,
                    )
                if ci == nch - 1:
                    emit_epilogue(bb)
            flush_outs(n_bb)
    install_legalizer(nc)
    return nc


_module_cache: dict = {}


def get_module() -> bass.Bass:
    if "nc" not in _module_cache:
        _module_cache["nc"] = build_module()
    return _module_cache["nc"]


def make_in_maps(inputs: dict) -> list[dict]:
    """Shard the full inputs into one input map per core (pure data parallel
    on the batch dim; tiny weights replicated).  x and v are staged in bf16
    (fp32 accumulation on device keeps the result inside the error gate);
    v is host-replicated to [128, L] so the on-device load is a plain
    full-rate DMA."""
    x = np.asarray(inputs["x"], dtype=np.float32)
    xb = np.ascontiguousarray(x).astype(ml_dtypes.bfloat16)
    w1 = np.asarray(inputs["w1"], dtype=np.float32)
    vb = np.ascontiguousarray(w1[0, :, 0]).astype(ml_dtypes.bfloat16)
    vtab = np.ascontiguousarray(np.broadcast_to(vb[None, :], (P, L)))
    s0 = float(sum(
        np.asarray(inputs[k], np.float32).reshape(-1)[0]
        for k in ("b1", "b2", "b3", "b4", "b5")
    ))
    wd_row = np.ascontiguousarray(np.asarray(inputs["wd"], np.float32)[0, :])
    bd = np.asarray(inputs["bd"], np.float32).reshape(-1)
    bd_eff = np.ascontiguousarray((s0 * wd_row + bd).astype(np.float32))

    maps = []
    for c in range(N_CORES):
        m = {"vtab": vtab, "wdrow": wd_row, "bdeff": bd_eff}
        base = c * B_CORE
        for bb in range(B_CORE // P):
            m[f"x{bb}"] = np.ascontiguousarray(
                xb[base + bb * P:base + (bb + 1) * P]
            )
        maps.append(m)
    return maps


def kernel(**inputs) -> np.ndarray:
    nc = get_module()
    in_maps = make_in_maps(inputs)
    res = run_bass_kernel_spmd(nc, in_maps, core_ids=list(range(N_CORES)))
    return np.concatenate([r["out"] for r in res.results], axis=0)


# revision 31
# speedup vs baseline: 1.1730x; 1.1329x over previous
"""Trainium2 Bass kernel for nn_AudioDeviceModel (dense_cnn, memory-bound).

The reference model applies a chain of dilated kernel-size-2 convs to a
length-1 sequence with SAME padding.  For dilation d the two taps land at
padded positions 0 and d while the real sample sits at position d//2, so
every conv after the first reduces to its bias; the first conv (dilation 1,
pad_low=0) reduces to tap 0: a dot product of x[b, :] with w1[0, :, 0].
The whole model is therefore

    out[b, j] = (x[b, :] . w1[0, :, 0]) * wd[0, j] + bd_eff[j]
    bd_eff[j] = (b1 + b2 + b3 + b4 + b5) * wd[0, j] + bd[j]

(verified numerically against the jax reference).  A pure memory-bound
row-wise dot product over a 512 MiB matrix, data-parallel over 8 cores.

x and v are staged to HBM in bf16 (fp32 accumulation on device; max rel
err ~2e-3 vs the 2e-2 gate), so per-core traffic is ~38 MB: 32 MiB of x,
a 4 MiB host-replicated v table ([128, L], a plain full-rate load -- the
stride-0 broadcast DMA measured ~170 GB/s and the PE rank-1 broadcast
~0.8 us per 512 cols, both worse), outputs and epilogue consts.

Engine facts measured on HW here: HWDGE rings burst 420+ GB/s when both
queues are fed; scalar_tensor_tensor and ACT activation run at 1x for
bf16; tensor_tensor runs at 2x.  No single engine keeps up with the
stream, so the multiply+reduce is split per chunk:

- 'stt' chunks: DVE scalar_tensor_tensor (1x, single hop).
- 'tt' chunks: DVE tensor_tensor multiply (2x) + ACT activation(Copy,
  accum_out) row-sum (1x).  DVE ~89 us, ACT ~85 us.
(GpSimd cannot help: the v3 ISA restricts Pool tensor_reduce and
TensorScalarPtr to int ops.)

Block 0 starts with 2048/4096 chunks so compute begins as soon as the
first v piece lands (~8 us); the last three row-blocks alternate stt/tt
at 4096 granularity so DVE and ACT drain the stream tail in parallel,
tapering to 512 so the post-last-byte tail is ~2 us.

DMA topology: x tiles alternate 1:1 between the sync and scalar HWDGE
rings (a 2:1 split starves one queue: a single queue sustains only
~280 GB/s; mixing the gpsimd SWDGE ring in measured 276 GB/s).  Every
chunk's DMA is issued PREFETCH chunks ahead of its compute in program
order so ring doorbells are never queued behind multi-us compute ops
(that convoy cost v5 ~25 us).  Out-DMAs are emitted on the sync ring
OUT_DELAY blocks late for the same reason.  The epilogue consts ride
the gpsimd SWDGE ring.

This container's walrus build only accepts ONE on_wait and ONE on_update
per instruction, while Tile emits multi-wait instructions (kernel-tail
drain, multi-dependency compute ops).  legalize_bir_sync() splits the
extras into standalone EventSemaphore/NoOp instructions on the same engine
(sequencers are in-order, so a wait immediately before an instruction is
equivalent; trailing updates only on non-DMA instructions).
"""

import json

import ml_dtypes
import numpy as np

import concourse.bass as bass
import concourse.mybir as mybir
import concourse.tile as tile
from concourse.bass_utils import run_bass_kernel_spmd

FP32 = mybir.dt.float32
BF16 = mybir.dt.bfloat16

N_CORES = 8
B_FULL = 8192
L = 16384
J = 128
B_CORE = B_FULL // N_CORES  # 1024
P = 128                     # SBUF partitions
F = 8192                    # max L-chunk (free dim) per DMA tile

PREFETCH = 7                # chunks of DMA lead over compute
OUT_DELAY = 3               # flush out-DMAs this many row-blocks late

# (offset, size, mode) per row-block; sizes sum to L.  'stt' = DVE
# scalar_tensor_tensor (1x); 'tt' = DVE tensor_tensor (2x) + ACT accum.
CHUNK_TABLE = [
    [(0, 2048, 'stt'), (2048, 2048, 'stt'), (4096, 4096, 'stt'),
     (8192, 4096, 'tt'), (12288, 4096, 'tt')],                       # 0
    [(0, 8192, 'tt'), (8192, 8192, 'tt')],                           # 1
    [(0, 4096, 'stt'), (4096, 4096, 'tt'), (8192, 8192, 'tt')],      # 2
    [(0, 8192, 'tt'), (8192, 8192, 'tt')],                           # 3
    [(0, 8192, 'tt'), (8192, 8192, 'tt')],                           # 4
    [(0, 4096, 'stt'), (4096, 4096, 'tt'),
     (8192, 4096, 'stt'), (12288, 4096, 'tt')],                      # 5
    [(0, 4096, 'tt'), (4096, 4096, 'stt'),
     (8192, 4096, 'tt'), (12288, 4096, 'stt')],                      # 6
    [(0, 4096, 'stt'), (4096, 4096, 'tt'), (8192, 2048, 'stt'),
     (10240, 2048, 'tt'), (12288, 1024, 'stt'), (13312, 1024, 'tt'),
     (14336, 512, 'stt'), (14848, 1024, 'tt'), (15872, 512, 'stt')],  # 7
]
for _chunks in CHUNK_TABLE:
    assert sum(c[1] for c in _chunks) == L


def legalize_bir_sync(bir_bytes: bytes) -> bytes:
    """Split >1 on_wait / on_update per instruction for this walrus build."""
    mod = json.loads(bir_bytes)
    for fn in mod["functions"]:
        for bb in fn["blocks"]:
            out = []
            for ins in bb["instructions"]:
                si = ins.get("sync_info")
                waits = (si or {}).get("on_wait") or []
                ups = (si or {}).get("on_update") or []
                if len(waits) > 1:
                    for i, w in enumerate(waits[:-1]):
                        out.append({
                            "debug": ins.get("debug"),
                            "engine": ins["engine"],
                            "ins": [],
                            "outs": [],
                            "name": f"{ins['name']}_lw{i}",
                            "opcode": "EventSemaphore",
                            "sync_info": {"on_update": [], "on_wait": [w]},
                        })
                    si["on_wait"] = [waits[-1]]
                out.append(ins)
                if len(ups) > 1:
                    if ins.get("opcode") == "DMACopy":
                        raise RuntimeError(
                            f"multi-update on DMA {ins['name']} cannot be legalized"
                        )
                    for i, u in enumerate(ups[1:]):
                        out.append({
                            "debug": ins.get("debug"),
                            "engine": ins["engine"],
                            "ins": [],
                            "outs": [],
                            "name": f"{ins['name']}_lu{i}",
                            "opcode": "NoOp",
                            "sync_info": {"on_update": [u], "on_wait": []},
                        })
                    si["on_update"] = [ups[0]]
            bb["instructions"] = out
    return json.dumps(mod).encode()


def install_legalizer(nc):
    orig = nc.to_json_bytes

    def patched():
        return legalize_bir_sync(orig())

    nc.to_json_bytes = patched
    return nc


def build_module(b_core: int = B_CORE, l: int = L) -> bass.Bass:
    n_bb = b_core // P
    nc = bass.Bass()
    x_ds = [
        nc.dram_tensor(f"x{bb}", [P, l], BF16, kind="ExternalInput")
        for bb in range(n_bb)
    ]
    v_d = nc.dram_tensor("vtab", [P, l], BF16, kind="ExternalInput")
    wd_d = nc.dram_tensor("wdrow", [J], FP32, kind="ExternalInput")
    bd_d = nc.dram_tensor("bdeff", [J], FP32, kind="ExternalInput")
    out_d = nc.dram_tensor("out", [b_core, J], FP32, kind="ExternalOutput")

    # flat chunk list: (bb, ci, off, f, mode, n_chunks_in_block)
    all_chunks = []
    for bb, chunks in enumerate(CHUNK_TABLE):
        for ci, (off, f, mode) in enumerate(chunks):
            all_chunks.append((bb, ci, off, f, mode, len(chunks)))
    n_idx = len(all_chunks)

    with tile.TileContext(nc) as tc:
        with (
            tc.tile_pool(name="consts", bufs=1) as consts,
            tc.tile_pool(name="xp", bufs=10) as xp,
            tc.tile_pool(name="accp", bufs=20) as accp,
            tc.tile_pool(name="outp", bufs=5) as outp,
        ):
            # 1:1 ring split: SDMA round-robins the queues byte-fairly, so
            # any byte imbalance turns the heavy queue into a straggler
            # (a 2:1 split measured ~287 GB/s vs ~420 for 1:1).
            rings = (nc.sync, nc.scalar)

            # v table (host-replicated to all 128 partitions): v[0:8192]
            # as two 1 MiB loads up front (block 0's chunks unblock ~8 us
            # in); v[8192:] interleaved behind the first x tiles below so
            # it doesn't delay their completions.
            v_full = consts.tile([P, l], BF16, name="v_full", tag="v_full")
            pending_v = []
            for r in range(4):
                def load_v(r=r):
                    rings[r % 2].dma_start(
                        out=v_full[:, r * 4096:(r + 1) * 4096],
                        in_=v_d[:, r * 4096:(r + 1) * 4096],
                    )
                if r < 2:
                    load_v()
                else:
                    pending_v.append(load_v)

            # tiny epilogue consts on the gpsimd (SWDGE) ring.
            wd_b = consts.tile([P, J], FP32, name="wd_b", tag="wd_b")
            nc.gpsimd.dma_start(
                out=wd_b, in_=wd_d[:].unsqueeze(0).partition_broadcast(P)
            )
            bd_b = consts.tile([P, J], FP32, name="bd_b", tag="bd_b")
            nc.gpsimd.dma_start(
                out=bd_b, in_=bd_d[:].unsqueeze(0).partition_broadcast(P)
            )

            x_tiles = {}
            accs = {}
            pending_outs = []   # (bb, o_t)

            def issue_dma(idx):
                bb, ci, off, f, mode, _ = all_chunks[idx]
                x_t = xp.tile([P, F], BF16, name="x_t", tag="x_t")
                x_tiles[idx] = x_t
                rings[idx % 2].dma_start(
                    out=x_t[:, :f], in_=x_ds[bb][:, off:off + f]
                )
                if pending_v:
                    pending_v.pop(0)()

            def flush_outs(upto_bb):
                while pending_outs and pending_outs[0][0] <= upto_bb:
                    obb, o_t = pending_outs.pop(0)
                    nc.sync.dma_start(
                        out=out_d[obb * P:(obb + 1) * P, :], in_=o_t
                    )

            def emit_epilogue(bb):
                t = accp.tile([P, 1], FP32, name=f"t{bb}", tag=f"t{bb}")
                nc.vector.tensor_reduce(
                    out=t, in_=accs[bb], axis=mybir.AxisListType.X,
                    op=mybir.AluOpType.add,
                )
                o_t = outp.tile([P, J], FP32)
                nc.vector.scalar_tensor_tensor(
                    out=o_t, in0=wd_b, scalar=t, in1=bd_b,
                    op0=mybir.AluOpType.mult, op1=mybir.AluOpType.add,
                )
                pending_outs.append((bb, o_t))

            for idx in range(min(PREFETCH, n_idx)):
                issue_dma(idx)

            for idx, (bb, ci, off, f, mode, nch) in enumerate(all_chunks):
                if idx + PREFETCH < n_idx:
                    nbb = all_chunks[idx + PREFETCH][0]
                    flush_outs(nbb - OUT_DELAY)
                    issue_dma(idx + PREFETCH)
                if bb not in accs:
                    accs[bb] = accp.tile(
                        [P, nch], FP32, name=f"acc{bb}", tag=f"acc{bb}"
                    )
                x_t = x_tiles.pop(idx)
                acc_col = accs[bb][:, ci:ci + 1]
                v_sl = v_full[:, off:off + f]
                if mode == 'stt':
                    nc.vector.scalar_tensor_tensor(
                        out=x_t[:, :f], in0=x_t[:, :f], scalar=1.0,
                        in1=v_sl,
                        op0=mybir.AluOpType.mult,
                        op1=mybir.AluOpType.mult,
                        accum_out=acc_col,
                    )
                else:
                    nc.vector.tensor_tensor(
                        out=x_t[:, :f], in0=x_t[:, :f],
                        in1=v_sl, op=mybir.AluOpType.mult,
                    )
                    nc.scalar.activation(
                        out=x_t[:, :f], in_=x_t[:, :f],
                        func=mybir.ActivationFunctionType.Copy,
                        accum_out=acc_col,
                    )
                if ci == nch - 1:
                    emit_epilogue(bb)
            flush_outs(n_bb)
    install_legalizer(nc)
    return nc


_module_cache: dict = {}


def get_module() -> bass.Bass:
    if "nc" not in _module_cache:
        _module_cache["nc"] = build_module()
    return _module_cache["nc"]


def make_in_maps(inputs: dict) -> list[dict]:
    """Shard the full inputs into one input map per core (pure data parallel
    on the batch dim; tiny weights replicated).  x and v are staged in bf16
    (fp32 accumulation on device keeps the result inside the error gate);
    v is host-replicated to [128, L] so the on-device load is a plain
    full-rate DMA."""
    x = np.asarray(inputs["x"], dtype=np.float32)
    xb = np.ascontiguousarray(x).astype(ml_dtypes.bfloat16)
    w1 = np.asarray(inputs["w1"], dtype=np.float32)
    vb = np.ascontiguousarray(w1[0, :, 0]).astype(ml_dtypes.bfloat16)
    vtab = np.ascontiguousarray(np.broadcast_to(vb[None, :], (P, L)))
    s0 = float(sum(
        np.asarray(inputs[k], np.float32).reshape(-1)[0]
        for k in ("b1", "b2", "b3", "b4", "b5")
    ))
    wd_row = np.ascontiguousarray(np.asarray(inputs["wd"], np.float32)[0, :])
    bd = np.asarray(inputs["bd"], np.float32).reshape(-1)
    bd_eff = np.ascontiguousarray((s0 * wd_row + bd).astype(np.float32))

    maps = []
    for c in range(N_CORES):
        m = {"vtab": vtab, "wdrow": wd_row, "bdeff": bd_eff}
        base = c * B_CORE
        for bb in range(B_CORE // P):
            m[f"x{bb}"] = np.ascontiguousarray(
                xb[base + bb * P:base + (bb + 1) * P]
            )
        maps.append(m)
    return maps


def kernel(**inputs) -> np.ndarray:
    nc = get_module()
    in_maps = make_in_maps(inputs)
    res = run_bass_kernel_spmd(nc, in_maps, core_ids=list(range(N_CORES)))
    return np.concatenate([r["out"] for r in res.results], axis=0)
